# revision 32
# baseline (speedup 1.0000x reference)
"""Trainium2 Bass kernel for MultiHeadLatentAttention (MLA), 8-core SPMD.

Sharding: data-parallel over batch (4) x tensor-parallel over heads (2).
Core c handles batch c//2 and heads (c%2)*8 .. +8. Each core computes its
partial output projection; the host sums the two TP partials per batch and
adds the output bias.

Device layout is feature-on-partition / token-on-free throughout, so every
projection is a plain matmul chain with no transposes. The only transposes
are the softmax-prob tiles (bf16, done on the DMA XBAR, not the PE).
"""

import sys
from contextlib import ExitStack

import numpy as np
import ml_dtypes

for _p in ("/opt/trn_rl_repo", "/root/.axon_site/_ro/trn_rl_repo"):
    if _p not in sys.path:
        sys.path.append(_p)

import concourse.bass as bass  # noqa: E402
import concourse.mybir as mybir  # noqa: E402
from concourse import bacc  # noqa: E402
from concourse.bass_utils import run_bass_kernel_spmd  # noqa: E402
from concourse.tile import TileContext  # noqa: E402

# Problem shapes (hardcoded per contract)
B, S, D = 4, 1024, 2048
H = 16
QL, KVL = 1536, 512
NOPE, ROPE, VH = 128, 64, 128
QKH = NOPE + ROPE  # 192
EPS = 1e-6

P = 128
T = S          # tokens per core (one batch)
DC = D // P    # 16 X chunks
QC = QL // P   # 12 q-latent chunks
KC = KVL // P  # 4 kv-latent chunks
HH = H // 2    # 8 heads per core
NKV = KVL + ROPE  # 576
NEG = -1.0e4   # mask bias (exp underflows to exactly 0)

f32 = mybir.dt.float32
f32r = mybir.dt.float32r
bf16 = mybir.dt.bfloat16


def _r(ap):
    """bitcast an fp32 AP to float32r for full-rate PE matmuls"""
    if ap.dtype == f32r:
        return ap
    return ap.bitcast(f32r)


def _nblk(start, qi):
    """number of 128-wide key blocks visible to query chunk qi"""
    n = (start + (qi + 1) * P + P - 1) // P
    return max(1, min(S // P, n))


def build_nc(start: int):
    nc = bacc.Bacc(None, target_bir_lowering=False, debug=False)

    dp = nc.declare_dram_parameter
    xt = dp("xt", [D, T], bf16, isOutput=False)           # X[b].T
    wqdl = dp("wqdl", [D, 6 * P], bf16, isOutput=False)   # local qd W.T
    wkvd = dp("wkvd", [D, 5 * P], bf16, isOutput=False)   # kv down W.T (padded)
    wqu = dp("wqu", [QL, QL], bf16, isOutput=False)        # (perm q) Wqu_eff.T
    wkvu = dp("wkvu", [KVL, 2048], bf16, isOutput=False)   # (perm kv) Wkvu_eff.T
    wo = dp("wo", [HH * VH, D], bf16, isOutput=False)     # Wo[:, slice].T
    bql_i = dp("bql", [P, 6], f32, isOutput=False)        # local qd bias
    bkvd_i = dp("bkvd", [P, 5], f32, isOutput=False)      # kv down bias
    bqu_i = dp("bqu", [P, QC], f32, isOutput=False)       # perm + scale
    bkvuk = dp("bkvuk", [P, HH], f32, isOutput=False)     # kNope part
    bkvuv = dp("bkvuv", [1, HH * VH], bf16, isOutput=False)  # v part row
    cos2 = dp("cos2", [P, T], f32, isOutput=False)        # duplicated rows
    sina = dp("sina", [P, T], f32, isOutput=False)        # sign-folded sin
    maskt = dp("maskt", [P, 8, 1024], bf16, isOutput=False)  # scoresT mask
    onescol = dp("onescol", [P, P], f32r, isOutput=False)
    outt = dp("outt", [D, T], f32, isOutput=True)

    cc_in = nc.dram_tensor("cc_in", [6 * P, T], bf16)
    cc_out = nc.dram_tensor("cc_out", [12 * P, T], bf16)
    RG = [[0, 1], [2, 3], [4, 5], [6, 7]]

    xt_r = xt.rearrange("(c p) t -> p c t", p=P)
    wqdl_r = wqdl.rearrange("(c p) m -> p c m", p=P)
    wkvd_r = wkvd.rearrange("(c p) m -> p c m", p=P)
    wqu_r = wqu.rearrange("(c p) m -> p c m", p=P)
    wkvu_r = wkvu.rearrange("(c p) m -> p c m", p=P)
    wo_r = wo.rearrange("(c p) m -> p c m", p=P)
    outt_r = outt.rearrange("(c p) t -> p c t", p=P)

    with TileContext(nc) as tc, ExitStack() as stk:
        const = stk.enter_context(tc.tile_pool(name="const", bufs=1))
        persist = stk.enter_context(tc.tile_pool(name="persist", bufs=1))
        psA = stk.enter_context(tc.tile_pool(name="psA", bufs=8, space="PSUM"))

        # ---- constants in SBUF ----
        c_bql = const.tile([P, 6], f32)
        nc.sync.dma_start(c_bql[:], bql_i[:])
        c_bkvd = const.tile([P, 5], f32)
        nc.sync.dma_start(c_bkvd[:], bkvd_i[:])
        c_bqu = const.tile([P, QC], f32)
        nc.sync.dma_start(c_bqu[:], bqu_i[:])
        c_bkvuk = const.tile([P, HH], f32)
        nc.sync.dma_start(c_bkvuk[:], bkvuk[:])
        c_bkvuv = const.tile([1, HH * VH], bf16)
        nc.sync.dma_start(c_bkvuv[:], bkvuv[:])
        c_cos = const.tile([P, T], f32)
        nc.sync.dma_start(c_cos[:], cos2[:])
        c_sin = const.tile([P, T], f32)
        nc.sync.dma_start(c_sin[:], sina[:])
        ones_col = const.tile([P, P], f32r)   # partition-sum + broadcast
        nc.sync.dma_start(ones_col[:], onescol[:])
        ones_row = const.tile([1, P], f32)    # lhsT for fp32 bcast matmul
        nc.vector.memset(ones_row[:], 1.0)
        ones_row_b = const.tile([1, P], bf16)  # lhsT for v-bias matmul
        nc.vector.memset(ones_row_b[:], 1.0)
        eps_t = const.tile([1, 1], f32)
        nc.vector.memset(eps_t[:], EPS)
        ones_bf = const.tile([P, P], bf16)
        nc.vector.memset(ones_bf[:], 1.0)
        eps_c = const.tile([P, 1], f32)
        nc.vector.memset(eps_c[:], EPS)

        # ---- persistent activations ----
        t_q = persist.tile([P, QC, T], bf16)      # q heads (nope 0-7, rope 8-11)
        t_kn = persist.tile([P, HH, T], bf16)     # kNope[feat, head, tok]
        t_v = persist.tile([P, T // P, HH * P], bf16)  # v[tok, tokchunk, hv]
        t_kr = persist.tile([P, T], bf16)         # kRot, rows duplicated
        t_ao = persist.tile([P, HH, T], bf16)     # attn out [vh, head, tok]

        # ====== phases 1+2 per 512-token half ======
        with tc.tile_pool(name="ph1", bufs=1) as ph1, \
             tc.tile_pool(name="wstream", bufs=3) as wst, \
             tc.tile_pool(name="wqu_p", bufs=3) as wqp, \
             tc.tile_pool(name="wkvu_p", bufs=3) as wkp, \
             tc.tile_pool(name="tmp", bufs=2) as tmp, \
             tc.tile_pool(name="stage", bufs=2) as stg:
            def ev512(fn):
                # run fn(tt, psum_tile) for the two 512-token tiles
                for tt in range(2):
                    ps = psA.tile([P, 512], f32, tag="ev", name="ps_ev")
                    fn(tt, ps)

            # ---- down projections (kv duplicated, qd split 6/6) ----
            t_x = ph1.tile([P, DC, T], bf16, name="t_x")
            for c in range(DC):
                eng = nc.sync if c % 2 == 0 else nc.scalar
                eng.dma_start(t_x[:, c, :], xt_r[:, c, :])
            t_kv = ph1.tile([P, 5, T], bf16, name="t_kv")
            t_qd = ph1.tile([P, QC, T], bf16, name="t_qd")

            def dgroup(wt, m_rows, bias_t, bcol, out_ap):
                def fn(tt, ps):
                    psm = ps[:m_rows, :]
                    for c in range(DC):
                        nc.tensor.matmul(
                            psm, wt[:, c, :m_rows],
                            t_x[:, c, bass.ts(tt, 512)],
                            start=(c == 0), stop=(c == DC - 1),
                        )
                    nc.scalar.add(out_ap[:m_rows, bass.ts(tt, 512)], psm,
                                  bias_t[:m_rows, bcol:bcol + 1])
                ev512(fn)

            cc_in_r = cc_in.rearrange("(c p) t -> p c t", p=P)
            for m in range(6):
                wt = wst.tile([P, DC, P], bf16, tag="wqd")
                eng = nc.scalar if m % 2 == 0 else nc.sync
                eng.dma_start(wt[:], wqdl_r[:, :, bass.ts(m, P)])
                dgroup(wt, P, c_bql, m, t_qd[:, m, :])
                nc.gpsimd.dma_start(cc_in_r[:, m, :], t_qd[:, m, :])
            nc.gpsimd.collective_compute(
                "AllGather", mybir.AluOpType.bypass,
                replica_groups=RG,
                ins=[cc_in[:]], outs=[cc_out[:]],
            )

            for m in range(4):
                wt = wst.tile([P, DC, P], bf16, tag="wqd")
                eng = nc.scalar if m % 2 == 0 else nc.sync
                eng.dma_start(wt[:], wkvd_r[:, :, bass.ts(m, P)])
                dgroup(wt, P, c_bkvd, m, t_kv[:, m, :])
            wt = wst.tile([P, DC, P], bf16, tag="wqd")
            nc.scalar.dma_start(wt[:, :, :ROPE],
                                wkvd_r[:, :, bass.ds(512, ROPE)])
            dgroup(wt, ROPE, c_bkvd, 4, t_kv[:, 4, :])

            def rms_scale(srct, nchunk, denom):
                # (128, T) tile of rsqrt(mean sq + eps), bcast over rows
                rbc = tmp.tile([P, T], f32, tag="rbc", bufs=1)

                def fn(tt, ps):
                    for c in range(nchunk):
                        sq = tmp.tile([P, 512], f32r, tag="sq")
                        nc.vector.tensor_mul(
                            sq[:], srct[:, c, bass.ts(tt, 512)],
                            srct[:, c, bass.ts(tt, 512)])
                        nc.tensor.matmul(
                            ps, ones_col[:], _r(sq[:]),
                            start=(c == 0), stop=(c == nchunk - 1),
                        )
                    nc.scalar.activation(
                        rbc[:, bass.ts(tt, 512)], ps,
                        mybir.ActivationFunctionType.Sqrt,
                        bias=eps_c[:], scale=1.0 / denom,
                    )
                ev512(fn)
                nc.vector.reciprocal(rbc[:], rbc[:])
                return rbc

            # ---- kv path (independent of the exchange) ----
            rkv = rms_scale(t_kv, KC, float(KVL))
            for c in range(KC):
                nc.vector.tensor_mul(t_kv[:, c, :], t_kv[:, c, :], rkv[:])
            # RoPE on k
            swp = tmp.tile([P, T], bf16, tag="swp", name="swp",
                           bufs=1)[:ROPE, :]
            nc.sync.dma_start(swp[0:32, :], t_kv[32:64, 4, :])
            nc.sync.dma_start(swp[32:64, :], t_kv[0:32, 4, :])
            nc.vector.tensor_mul(t_kr[0:ROPE, :], t_kv[0:ROPE, 4, :],
                                 c_cos[0:ROPE, :])
            nc.vector.tensor_mul(swp[:], swp[:], c_sin[0:ROPE, :])
            nc.vector.tensor_add(t_kr[0:ROPE, :], t_kr[0:ROPE, :], swp[:])
            nc.sync.dma_start(t_kr[ROPE:P, :], t_kr[0:ROPE, :])
            # kNope up-projection
            for m in range(HH):
                wt = wkp.tile([P, KC, P], bf16, tag="wkn")
                (nc.scalar if m % 2 == 0 else nc.sync).dma_start(
                    wt[:], wkvu_r[:, :, bass.ts(m, P)])

                def fn(tt, ps, m=m, wt=wt):
                    for c in range(KC):
                        nc.tensor.matmul(
                            ps, wt[:, c, :],
                            t_kv[:, c, bass.ts(tt, 512)],
                            start=(c == 0), stop=(c == KC - 1),
                        )
                    nc.scalar.add(
                        t_kn[:, m, bass.ts(tt, 512)], ps,
                        c_bkvuk[:, m:m + 1],
                    )
                ev512(fn)
            # v up-projection (token-on-partition)
            for g in range(2):
                wt = wkp.tile([P, KC, 512], bf16, tag="wv")
                nc.scalar.dma_start(
                    wt[:], wkvu_r[:, :, bass.ds(1024 + g * 512, 512)])
                for tcb in range(8):
                    ps = psA.tile([P, 512], f32, tag="ev", name="ps_ev")
                    for c in range(KC):
                        nc.tensor.matmul(
                            ps,
                            t_kv[:, c, bass.ts(tcb, P)],
                            wt[:, c, :],
                            start=(c == 0), stop=False,
                        )
                    nc.tensor.matmul(
                        ps,
                        ones_row_b[:],
                        c_bkvuv[:, bass.ts(g, 512)],
                        start=False, stop=True,
                    )
                    nc.scalar.copy(t_v[:, tcb, bass.ts(g, 512)], ps)

            # ---- q path (after the exchange) ----
            co = cc_out.rearrange("(c p) t -> p c t", p=P)
            nc.sync.dma_start(t_qd[:, 0:6, :], co[:, 0:6, :])
            nc.scalar.dma_start(t_qd[:, 6:12, :], co[:, 6:12, :])
            rq = rms_scale(t_qd, QC, float(QL))
            for m in range(QC):
                wt = wqp.tile([P, QC, P], bf16, tag="wqu")
                eng = nc.scalar if m % 2 == 0 else nc.sync
                eng.dma_start(wt[:], wqu_r[:, :, bass.ts(m, P)])

                def fn(tt, ps, m=m, wt=wt):
                    tsl = bass.ts(tt, 512)
                    for c in range(QC):
                        nc.tensor.matmul(
                            ps, wt[:, c, :], t_qd[:, c, tsl],
                            start=(c == 0), stop=(c == QC - 1),
                        )
                    if m < 8:
                        qsb = stg.tile([P, 512], f32, tag="qsb", bufs=2)
                        nc.vector.tensor_mul(qsb[:], ps, rq[:, tsl])
                        nc.vector.tensor_scalar_add(
                            out=t_q[:, m, tsl], in0=qsb,
                            scalar1=c_bqu[:, m:m + 1],
                        )
                    else:
                        sq = stg.tile([P, 512], f32, tag="ropestage",
                                      bufs=2)
                        nc.vector.tensor_mul(sq[:], ps, rq[:, tsl])
                        nc.vector.tensor_scalar_add(
                            out=sq[:], in0=sq, scalar1=c_bqu[:, m:m + 1],
                        )
                        swq = stg.tile([P, 512], f32, tag="ropeswap",
                                       bufs=1)
                        for r0 in (0, 64):
                            nc.sync.dma_start(swq[r0:r0 + 32, :],
                                              sq[r0 + 32:r0 + 64, :])
                            nc.sync.dma_start(swq[r0 + 32:r0 + 64, :],
                                              sq[r0:r0 + 32, :])
                        nc.vector.tensor_mul(sq[:], sq[:], c_cos[:, tsl])
                        nc.vector.tensor_mul(swq[:], swq[:], c_sin[:, tsl])
                        nc.vector.tensor_add(sq[:], sq[:], swq[:])
                        nc.vector.tensor_copy(t_q[:, m, tsl], sq[:])
                ev512(fn)

        # ====== phase 3: attention (transposed scores, max-free) ======
        # scoresT[k, q] = kf . qf ; exp without row-max (scores are small);
        # denominator via ones-matmul over k partitions; normalize at end.
        def vis_qts(kc):
            return [qt for qt in range(2)
                    if qt * 512 + 511 >= kc * P - start]

        with tc.tile_pool(name="att", bufs=2) as att, \
             tc.tile_pool(name="attc", bufs=1) as attc:
            c_maskt = attc.tile([P, 8, 1024], bf16)
            nc.scalar.dma_start(c_maskt[:], maskt[:])
            # which (kc, qt) score tiles contain masked entries
            mask_tiles = set()
            for kc in range(8):
                for qt in vis_qts(kc):
                    lo, hi = qt * 512, qt * 512 + 512
                    # masked iff some q in tile has kc*P+127 > start+q
                    if kc * P + 127 > start + lo:
                        mask_tiles.add((kc, qt))
            for h in range(HH):
                rc = 8 + h // 2          # rope chunk for this head
                r0 = (h % 2) * ROPE      # rope partition base
                expt = att.tile([P, 8, T], bf16, tag="expt", name="expt")
                outU = {qt: psA.tile([P, 512], f32, tag="ev", name="outU")
                        for qt in range(2)}
                den = {qt: psA.tile([P, 512], f32, tag="ev", name="den")
                       for qt in range(2)}
                nvis = {qt: [kc for kc in range(8) if qt in vis_qts(kc)]
                        for qt in range(2)}

                prev = None
                for kc in range(8):
                    for qt in vis_qts(kc):
                        qsl = bass.ts(qt, 512)
                        sc = psA.tile([P, 512], f32, tag="ev", name="sc")
                        nc.tensor.matmul(
                            sc, t_kn[:, h, bass.ts(kc, P)], t_q[:, h, qsl],
                            start=True, stop=False,
                        )
                        nc.tensor.matmul(
                            sc,
                            t_kr[r0:r0 + ROPE, bass.ts(kc, P)],
                            t_q[r0:r0 + ROPE, rc, qsl],
                            start=False, stop=True,
                        )
                        if (kc, qt) in mask_tiles:
                            nc.vector.tensor_add(
                                sc[:], sc[:], c_maskt[:, kc, qsl])
                        nc.scalar.activation(
                            expt[:, kc, qsl], sc[:],
                            mybir.ActivationFunctionType.Exp,
                        )
                # contiguous PSUM accumulation groups (den / outU per qt)
                for qt in range(2):
                    kcs = nvis[qt]
                    for i, kc in enumerate(kcs):
                        nc.tensor.matmul(
                            den[qt], ones_bf[:],
                            expt[:, kc, bass.ts(qt, 512)],
                            start=(i == 0), stop=(i == len(kcs) - 1),
                        )
                    for i, kc in enumerate(kcs):
                        nc.tensor.matmul(
                            outU[qt], t_v[:, kc, bass.ts(h, P)],
                            expt[:, kc, bass.ts(qt, 512)],
                            start=(i == 0), stop=(i == len(kcs) - 1),
                        )
                for qt in range(2):
                    rcp = att.tile([P, 512], f32, tag="rcp", name="rcp")
                    nc.vector.reciprocal(rcp[:], den[qt])
                    nc.vector.tensor_mul(
                        t_ao[:, h, bass.ts(qt, 512)], outU[qt], rcp[:])

        # ====== phase 4: output projection ======
        with tc.tile_pool(name="wo_p", bufs=2) as wop, \
             tc.tile_pool(name="outp", bufs=3) as outp:
            for m in range(DC):
                wt = wop.tile([P, HH, P], bf16, tag="wo")
                nc.scalar.dma_start(wt[:], wo_r[:, :, bass.ts(m, P)])
                for tt in range(2):
                    ps = psA.tile([P, 512], f32, tag="ev")
                    for c in range(HH):
                        nc.tensor.matmul(
                            ps, wt[:, c, :], t_ao[:, c, bass.ts(tt, 512)],
                            start=(c == 0), stop=(c == HH - 1),
                        )
                    ot = outp.tile([P, 512], f32, tag="ot")
                    nc.vector.tensor_copy(ot[:], ps)
                    nc.sync.dma_start(outt_r[:, m, bass.ts(tt, 512)], ot[:])

    nc.compile()
    return nc


_CACHE = {}


def _get_nc(start: int):
    if start not in _CACHE:
        _CACHE[start] = build_nc(start)
    return _CACHE[start]


def _prep_inputs(X, base_freq, Wqd, bqd, gq, Wqu, bqu, Wkv, bkv, gkv,
                 Wkvu, bkvu, Wo, bo, start):
    f = np.float32
    X = np.asarray(X, f)
    base_freq = np.asarray(base_freq, f)
    Wqd = np.asarray(Wqd, f); bqd = np.asarray(bqd, f)
    gq = np.asarray(gq, f); Wqu = np.asarray(Wqu, f); bqu = np.asarray(bqu, f)
    Wkv = np.asarray(Wkv, f); bkv = np.asarray(bkv, f)
    gkv = np.asarray(gkv, f); Wkvu = np.asarray(Wkvu, f)
    bkvu = np.asarray(bkvu, f)
    Wo = np.asarray(Wo, f); bo = np.asarray(bo, f)
    start = int(np.asarray(start).item())
    assert start >= 0

    scale = QKH ** (-0.5)
    bf = ml_dtypes.bfloat16

    # qd down W split 6/6 across the TP pair; kv down duplicated
    wqd_t = Wqd.T.astype(f)                                   # (D, QL)
    wkv_t = Wkv.T.astype(f)                                   # (D, NKV)
    wqdl, bql = [], []
    for g in range(2):
        wqdl.append(np.ascontiguousarray(
            wqd_t[:, g * 768:(g + 1) * 768]).astype(bf))
        bql.append(np.ascontiguousarray(
            bqd[g * 768:(g + 1) * 768].reshape(6, P).T))
    wkvd = np.concatenate([wkv_t[:, :576], np.zeros((D, 64), f)], 1)
    wkvd = np.ascontiguousarray(wkvd).astype(bf)
    bkvd_p = np.zeros((5 * P,), f); bkvd_p[:NKV] = bkv
    bkvd = np.ascontiguousarray(bkvd_p.reshape(5, P).T)

    ang = base_freq[:S]                                       # (S, ROPE)
    cos = np.ascontiguousarray(np.cos(ang).T.astype(f))       # (ROPE, S)
    sin = np.ascontiguousarray(np.sin(ang).T.astype(f))
    cos2 = np.ascontiguousarray(np.concatenate([cos, cos], 0))  # (128, S)
    sgn = np.ones((ROPE, 1), f); sgn[:ROPE // 2] = -1.0
    sins = sin * sgn                                          # sign-folded
    sina = np.ascontiguousarray(np.concatenate([sins, sins], 0))

    # transposed additive mask: maskt[k_local, kc, q] for scoresT tiles
    bfd = ml_dtypes.bfloat16
    maskt = np.zeros((8, P, S), np.float32)
    q_glob = np.arange(S)
    for kc in range(8):
        k_glob = kc * P + np.arange(P)
        vis = k_glob[:, None] <= (start + q_glob[None, :])
        maskt[kc] = np.where(vis, 0.0, NEG)
    maskt = np.ascontiguousarray(maskt.transpose(1, 0, 2)).astype(bfd)

    # per head-group tensors
    perm_q = np.concatenate(
        [np.arange(h * QKH, h * QKH + NOPE) for h in range(HH)]
        + [np.arange(h * QKH + NOPE, (h + 1) * QKH) for h in range(HH)]
    )
    perm_kv = np.concatenate(
        [np.arange(h * (NOPE + VH), h * (NOPE + VH) + NOPE) for h in range(HH)]
        + [np.arange(h * (NOPE + VH) + NOPE, (h + 1) * (NOPE + VH))
           for h in range(HH)]
    )
    gmaps = []
    for g in range(2):
        rq = slice(g * HH * QKH, (g + 1) * HH * QKH)
        rkv = slice(g * HH * (NOPE + VH), (g + 1) * HH * (NOPE + VH))
        wqu_g = (Wqu[rq, :] * gq[None, :] * scale)[perm_q]    # (1536, QL)
        bqu_g = (bqu[rq] * scale)[perm_q]
        wkvu_g = (Wkvu[rkv, :] * gkv[None, :])[perm_kv]       # (2048, KVL)
        bkvu_g = bkvu[rkv][perm_kv]
        wo_g = Wo[:, g * HH * VH:(g + 1) * HH * VH]           # (D, 1024)
        gmaps.append({
            "wqu": np.ascontiguousarray(wqu_g.T).astype(bf),
            "bqu": np.ascontiguousarray(bqu_g.reshape(QC, P).T),
            "wkvu": np.ascontiguousarray(wkvu_g.T).astype(bf),
            "bkvuk": np.ascontiguousarray(
                bkvu_g[:HH * NOPE].reshape(HH, P).T),
            "bkvuv": np.ascontiguousarray(
                bkvu_g[HH * NOPE:].reshape(1, HH * VH)).astype(bf),
            "wo": np.ascontiguousarray(wo_g.T).astype(bf),    # (1024, D)
        })

    xts = [np.ascontiguousarray(X[b].T).astype(bf) for b in range(B)]

    in_maps = []
    for c in range(8):
        b, g = c // 2, c % 2
        m = {
            "xt": xts[b], "wqdl": wqdl[g], "bql": bql[g],
            "wkvd": wkvd, "bkvd": bkvd,
            "cos2": cos2, "sina": sina, "maskt": maskt,
            "onescol": np.ones((P, P), f),
        }
        m.update(gmaps[g])
        in_maps.append(m)
    return in_maps, bo, start


def kernel(**inputs) -> np.ndarray:
    in_maps, bo, start = _prep_inputs(**inputs)
    nc = _get_nc(start)
    res = run_bass_kernel_spmd(nc, in_maps, core_ids=list(range(8)))
    out = np.empty((B, S, D), np.float32)
    for b in range(B):
        acc = res.results[2 * b]["outt"] + res.results[2 * b + 1]["outt"]
        out[b] = acc.T + bo[None, :]
    return out


# revision 36
# speedup vs baseline: 1.2084x; 1.2084x over previous
"""Trainium2 Bass kernel for MultiHeadLatentAttention (MLA), 8-core SPMD.

Sharding: data-parallel over batch (4) x tensor-parallel over heads (2).
Core c handles batch c//2 and heads (c%2)*8 .. +8. Each core computes its
partial output projection; the host sums the two TP partials per batch and
adds the output bias.

Device layout is feature-on-partition / token-on-free throughout, so every
projection is a plain matmul chain with no transposes. The only transposes
are the softmax-prob tiles (bf16, done on the DMA XBAR, not the PE).
"""

import sys
from contextlib import ExitStack

import numpy as np
import ml_dtypes

for _p in ("/opt/trn_rl_repo", "/root/.axon_site/_ro/trn_rl_repo"):
    if _p not in sys.path:
        sys.path.append(_p)

import concourse.bass as bass  # noqa: E402
import concourse.mybir as mybir  # noqa: E402
from concourse import bacc  # noqa: E402
from concourse.bass_utils import run_bass_kernel_spmd  # noqa: E402
from concourse.tile import TileContext  # noqa: E402

# Problem shapes (hardcoded per contract)
B, S, D = 4, 1024, 2048
H = 16
QL, KVL = 1536, 512
NOPE, ROPE, VH = 128, 64, 128
QKH = NOPE + ROPE  # 192
EPS = 1e-6

P = 128
T = S          # tokens per core (one batch)
DC = D // P    # 16 X chunks
QC = QL // P   # 12 q-latent chunks
KC = KVL // P  # 4 kv-latent chunks
HH = H // 2    # 8 heads per core
NKV = KVL + ROPE  # 576
NEG = -1.0e4   # mask bias (exp underflows to exactly 0)

f32 = mybir.dt.float32
f32r = mybir.dt.float32r
bf16 = mybir.dt.bfloat16


def _r(ap):
    """bitcast an fp32 AP to float32r for full-rate PE matmuls"""
    if ap.dtype == f32r:
        return ap
    return ap.bitcast(f32r)


def _nblk(start, qi):
    """number of 128-wide key blocks visible to query chunk qi"""
    n = (start + (qi + 1) * P + P - 1) // P
    return max(1, min(S // P, n))


def build_nc(start: int):
    nc = bacc.Bacc(None, target_bir_lowering=False, debug=False)

    dp = nc.declare_dram_parameter
    xt = dp("xt", [D, T], bf16, isOutput=False)           # X[b].T
    wqdl = dp("wqdl", [D, 6 * P], bf16, isOutput=False)   # local qd W.T
    wkvd = dp("wkvd", [D, 5 * P], bf16, isOutput=False)   # kv down W.T (padded)
    wqu = dp("wqu", [QL, QL], bf16, isOutput=False)        # (perm q) Wqu_eff.T
    wkvu = dp("wkvu", [KVL, 2048], bf16, isOutput=False)   # (perm kv) Wkvu_eff.T
    wo = dp("wo", [HH * VH, D], bf16, isOutput=False)     # Wo[:, slice].T
    bql_i = dp("bql", [P, 6], f32, isOutput=False)        # local qd bias
    bkvd_i = dp("bkvd", [P, 5], f32, isOutput=False)      # kv down bias
    bqu_i = dp("bqu", [P, QC], f32, isOutput=False)       # perm + scale
    bkvuk = dp("bkvuk", [P, HH], f32, isOutput=False)     # kNope part
    bkvuv = dp("bkvuv", [1, HH * VH], bf16, isOutput=False)  # v part row
    cos2 = dp("cos2", [P, T], f32, isOutput=False)        # duplicated rows
    sina = dp("sina", [P, T], f32, isOutput=False)        # sign-folded sin
    maskt = dp("maskt", [P, 8, 1024], bf16, isOutput=False)  # scoresT mask
    onescol = dp("onescol", [P, P], f32r, isOutput=False)
    outt = dp("outt", [D, T], f32, isOutput=True)

    cc_in = nc.dram_tensor("cc_in", [6 * P, T], bf16)
    cc_out = nc.dram_tensor("cc_out", [12 * P, T], bf16)
    RG = [[0, 1], [2, 3], [4, 5], [6, 7]]

    xt_r = xt.rearrange("(c p) t -> p c t", p=P)
    wqdl_r = wqdl.rearrange("(c p) m -> p c m", p=P)
    wkvd_r = wkvd.rearrange("(c p) m -> p c m", p=P)
    wqu_r = wqu.rearrange("(c p) m -> p c m", p=P)
    wkvu_r = wkvu.rearrange("(c p) m -> p c m", p=P)
    wo_r = wo.rearrange("(c p) m -> p c m", p=P)
    outt_r = outt.rearrange("(c p) t -> p c t", p=P)

    with TileContext(nc) as tc, ExitStack() as stk:
        const = stk.enter_context(tc.tile_pool(name="const", bufs=1))
        persist = stk.enter_context(tc.tile_pool(name="persist", bufs=1))
        psA = stk.enter_context(tc.tile_pool(name="psA", bufs=8, space="PSUM"))

        # ---- constants in SBUF ----
        c_bql = const.tile([P, 6], f32)
        nc.sync.dma_start(c_bql[:], bql_i[:])
        c_bkvd = const.tile([P, 5], f32)
        nc.sync.dma_start(c_bkvd[:], bkvd_i[:])
        c_bqu = const.tile([P, QC], f32)
        nc.sync.dma_start(c_bqu[:], bqu_i[:])
        c_bkvuk = const.tile([P, HH], f32)
        nc.sync.dma_start(c_bkvuk[:], bkvuk[:])
        c_bkvuv = const.tile([1, HH * VH], bf16)
        nc.sync.dma_start(c_bkvuv[:], bkvuv[:])
        c_cos = const.tile([P, T], f32)
        nc.scalar.dma_start(c_cos[:], cos2[:])
        c_sin = const.tile([P, T], f32)
        nc.scalar.dma_start(c_sin[:], sina[:])
        ones_col = const.tile([P, P], f32r)   # partition-sum + broadcast
        nc.sync.dma_start(ones_col[:], onescol[:])
        ones_row_b = const.tile([1, P], bf16)  # lhsT for v-bias matmul
        nc.vector.memset(ones_row_b[:], 1.0)
        ones_bf = const.tile([P, P], bf16)
        nc.vector.memset(ones_bf[:], 1.0)
        eps_c = const.tile([P, 1], f32)
        nc.vector.memset(eps_c[:], EPS)

        # ---- persistent activations ----
        t_q = persist.tile([P, QC, T], bf16)      # q heads (nope 0-7, rope 8-11)
        t_kn = persist.tile([P, HH, T], bf16)     # kNope[feat, head, tok]
        t_v = persist.tile([P, T // P, HH * P], bf16)  # v[tok, tokchunk, hv]
        t_kr = persist.tile([P, T], bf16)         # kRot, rows duplicated
        t_ao = persist.tile([P, HH, T], bf16)     # attn out [vh, head, tok]

        # ====== phases 1+2 per 512-token half ======
        with tc.tile_pool(name="ph1", bufs=1) as ph1, \
             tc.tile_pool(name="wstream", bufs=3) as wst, \
             tc.tile_pool(name="wqu_p", bufs=3) as wqp, \
             tc.tile_pool(name="wkvu_p", bufs=3) as wkp, \
             tc.tile_pool(name="tmp", bufs=2) as tmp, \
             tc.tile_pool(name="stage", bufs=2) as stg:
            def ev512(fn):
                # run fn(tt, psum_tile) for the two 512-token tiles
                for tt in range(2):
                    ps = psA.tile([P, 512], f32, tag="ev", name="ps_ev")
                    fn(tt, ps)

            # ---- down projections (kv duplicated, qd split 6/6) ----
            t_x = ph1.tile([P, DC, T], bf16, name="t_x")
            for c in range(DC):
                eng = nc.sync if c % 2 == 0 else nc.scalar
                eng.dma_start(t_x[:, c, :], xt_r[:, c, :])
            t_kv = ph1.tile([P, 5, T], bf16, name="t_kv")
            t_qd = ph1.tile([P, QC, T], bf16, name="t_qd")

            def dgroup(wt, m_rows, bias_t, bcol, out_ap):
                def fn(tt, ps):
                    psm = ps[:m_rows, :]
                    for c in range(DC):
                        nc.tensor.matmul(
                            psm, wt[:, c, :m_rows],
                            t_x[:, c, bass.ts(tt, 512)],
                            start=(c == 0), stop=(c == DC - 1),
                        )
                    nc.vector.tensor_scalar_add(
                        out=out_ap[:m_rows, bass.ts(tt, 512)], in0=psm,
                        scalar1=bias_t[:m_rows, bcol:bcol + 1])
                ev512(fn)

            cc_in_r = cc_in.rearrange("(c p) t -> p c t", p=P)
            for m in range(6):
                wt = wst.tile([P, DC, P], bf16, tag="wqd")
                eng = nc.scalar if m % 2 == 0 else nc.sync
                eng.dma_start(wt[:], wqdl_r[:, :, bass.ts(m, P)])
                dgroup(wt, P, c_bql, m, t_qd[:, m, :])
                nc.gpsimd.dma_start(cc_in_r[:, m, :], t_qd[:, m, :])
            nc.gpsimd.collective_compute(
                "AllGather", mybir.AluOpType.bypass,
                replica_groups=RG,
                ins=[cc_in[:]], outs=[cc_out[:]],
            )

            for m in range(4):
                wt = wst.tile([P, DC, P], bf16, tag="wqd")
                eng = nc.scalar if m % 2 == 0 else nc.sync
                eng.dma_start(wt[:], wkvd_r[:, :, bass.ts(m, P)])
                dgroup(wt, P, c_bkvd, m, t_kv[:, m, :])
            wt = wst.tile([P, DC, P], bf16, tag="wqd")
            nc.scalar.dma_start(wt[:, :, :ROPE],
                                wkvd_r[:, :, bass.ds(512, ROPE)])
            dgroup(wt, ROPE, c_bkvd, 4, t_kv[:, 4, :])

            def rms_scale(srct, nchunk, denom):
                # (128, T) tile of rsqrt(mean sq + eps), bcast over rows
                rbc = tmp.tile([P, T], f32, tag="rbc", bufs=1)

                def fn(tt, ps):
                    for c in range(nchunk):
                        sq = tmp.tile([P, 512], f32r, tag="sq")
                        nc.vector.tensor_mul(
                            sq[:], srct[:, c, bass.ts(tt, 512)],
                            srct[:, c, bass.ts(tt, 512)])
                        nc.tensor.matmul(
                            ps, ones_col[:], _r(sq[:]),
                            start=(c == 0), stop=(c == nchunk - 1),
                        )
                    nc.scalar.activation(
                        rbc[:, bass.ts(tt, 512)], ps,
                        mybir.ActivationFunctionType.Sqrt,
                        bias=eps_c[:], scale=1.0 / denom,
                    )
                ev512(fn)
                nc.vector.reciprocal(rbc[:], rbc[:])
                return rbc

            # ---- kv path (independent of the exchange) ----
            rkv = rms_scale(t_kv, KC, float(KVL))
            for c in range(KC):
                nc.vector.tensor_mul(t_kv[:, c, :], t_kv[:, c, :], rkv[:])
            # RoPE on k
            swp = tmp.tile([P, T], bf16, tag="swp", name="swp",
                           bufs=1)[:ROPE, :]
            nc.sync.dma_start(swp[0:32, :], t_kv[32:64, 4, :])
            nc.sync.dma_start(swp[32:64, :], t_kv[0:32, 4, :])
            nc.vector.tensor_mul(t_kr[0:ROPE, :], t_kv[0:ROPE, 4, :],
                                 c_cos[0:ROPE, :])
            nc.vector.tensor_mul(swp[:], swp[:], c_sin[0:ROPE, :])
            nc.vector.tensor_add(t_kr[0:ROPE, :], t_kr[0:ROPE, :], swp[:])
            nc.sync.dma_start(t_kr[ROPE:P, :], t_kr[0:ROPE, :])
            # kNope up-projection
            for m in range(HH):
                wt = wkp.tile([P, KC, P], bf16, tag="wkn")
                (nc.scalar if m % 2 == 0 else nc.sync).dma_start(
                    wt[:], wkvu_r[:, :, bass.ts(m, P)])

                def fn(tt, ps, m=m, wt=wt):
                    for c in range(KC):
                        nc.tensor.matmul(
                            ps, wt[:, c, :],
                            t_kv[:, c, bass.ts(tt, 512)],
                            start=(c == 0), stop=(c == KC - 1),
                        )
                    nc.vector.tensor_scalar_add(
                        out=t_kn[:, m, bass.ts(tt, 512)], in0=ps,
                        scalar1=c_bkvuk[:, m:m + 1])
                ev512(fn)
            # v up-projection (token-on-partition)
            for g in range(2):
                wt = wkp.tile([P, KC, 512], bf16, tag="wv")
                nc.scalar.dma_start(
                    wt[:], wkvu_r[:, :, bass.ds(1024 + g * 512, 512)])
                for tcb in range(8):
                    ps = psA.tile([P, 512], f32, tag="ev", name="ps_ev")
                    for c in range(KC):
                        nc.tensor.matmul(
                            ps,
                            t_kv[:, c, bass.ts(tcb, P)],
                            wt[:, c, :],
                            start=(c == 0), stop=False,
                        )
                    nc.tensor.matmul(
                        ps,
                        ones_row_b[:],
                        c_bkvuv[:, bass.ts(g, 512)],
                        start=False, stop=True,
                    )
                    nc.vector.tensor_copy(t_v[:, tcb, bass.ts(g, 512)], ps)

            # ---- q path (after the exchange) ----
            co = cc_out.rearrange("(c p) t -> p c t", p=P)
            nc.sync.dma_start(t_qd[:, 0:6, :], co[:, 0:6, :])
            nc.scalar.dma_start(t_qd[:, 6:12, :], co[:, 6:12, :])
            rq = rms_scale(t_qd, QC, float(QL))
            for m in (0, 1, 8, 2, 3, 9, 4, 5, 10, 6, 7, 11):
                wt = wqp.tile([P, QC, P], bf16, tag="wqu")
                eng = nc.scalar if m % 2 == 0 else nc.sync
                eng.dma_start(wt[:], wqu_r[:, :, bass.ts(m, P)])

                def fn(tt, ps, m=m, wt=wt):
                    tsl = bass.ts(tt, 512)
                    for c in range(QC):
                        nc.tensor.matmul(
                            ps, wt[:, c, :], t_qd[:, c, tsl],
                            start=(c == 0), stop=(c == QC - 1),
                        )
                    if m < 8:
                        qsb = stg.tile([P, 512], f32, tag="qsb", bufs=2)
                        nc.vector.tensor_mul(qsb[:], ps, rq[:, tsl])
                        nc.vector.tensor_scalar_add(
                            out=t_q[:, m, tsl], in0=qsb,
                            scalar1=c_bqu[:, m:m + 1],
                        )
                    else:
                        sq = stg.tile([P, 512], f32, tag="ropestage",
                                      bufs=2)
                        nc.vector.tensor_mul(sq[:], ps, rq[:, tsl])
                        nc.vector.tensor_scalar_add(
                            out=sq[:], in0=sq, scalar1=c_bqu[:, m:m + 1],
                        )
                        swq = stg.tile([P, 512], f32, tag="ropeswap",
                                       bufs=1)
                        for r0 in (0, 64):
                            nc.sync.dma_start(swq[r0:r0 + 32, :],
                                              sq[r0 + 32:r0 + 64, :])
                            nc.sync.dma_start(swq[r0 + 32:r0 + 64, :],
                                              sq[r0:r0 + 32, :])
                        nc.vector.tensor_mul(sq[:], sq[:], c_cos[:, tsl])
                        nc.vector.tensor_mul(swq[:], swq[:], c_sin[:, tsl])
                        nc.vector.tensor_add(sq[:], sq[:], swq[:])
                        nc.vector.tensor_copy(t_q[:, m, tsl], sq[:])
                ev512(fn)

        # ====== phase 3: attention (transposed scores, max-free) ======
        # scoresT[k, q] = kf . qf ; exp without row-max (scores are small);
        # denominator via ones-matmul over k partitions; normalize at end.
        def vis_qts(kc):
            return [qt for qt in range(2)
                    if qt * 512 + 511 >= kc * P - start]

        with tc.tile_pool(name="att", bufs=2) as att, \
             tc.tile_pool(name="attc", bufs=1) as attc:
            c_maskt = attc.tile([P, 8, 1024], bf16)
            nc.sync.dma_start(c_maskt[:], maskt[:])
            # which (kc, qt) score tiles contain masked entries
            nvis = {qt: [kc for kc in range(8) if qt in vis_qts(kc)]
                    for qt in range(2)}
            mask_tiles = set()
            for kc in range(8):
                for qt in vis_qts(kc):
                    lo, hi = qt * 512, qt * 512 + 512
                    # masked iff some q in tile has kc*P+127 > start+q
                    if kc * P + 127 > start + lo:
                        mask_tiles.add((kc, qt))
            for hp in range(4):
                h0, h1 = 2 * hp, 2 * hp + 1
                rc = 8 + hp              # rope chunk holds both heads
                expts = {h0: att.tile([P, 8, T], bf16, tag="expt",
                                      name="expt0"),
                         h1: att.tile([P, 8, T], bf16, tag="expt",
                                      name="expt1")}
                for kc in range(8):
                    for qt in vis_qts(kc):
                        lo = max(qt * 512, kc * P - start)
                        w = qt * 512 + 512 - lo
                        rel = lo - qt * 512
                        qsl = bass.ds(lo, w)
                        scs = {}
                        for h in (h0, h1):
                            sc = psA.tile([P, 512], f32, tag="ev",
                                          name="sc")
                            scs[h] = sc
                            nc.tensor.matmul(
                                sc[:, rel:], t_kn[:, h, bass.ts(kc, P)],
                                t_q[:, h, qsl],
                                start=True, stop=False,
                            )
                        # rope matmuls on disjoint PE row groups (packed)
                        for h in (h0, h1):
                            r0 = (h % 2) * ROPE
                            nc.tensor.matmul(
                                scs[h][:, rel:],
                                t_kr[r0:r0 + ROPE, bass.ts(kc, P)],
                                t_q[r0:r0 + ROPE, rc, qsl],
                                start=False, stop=True,
                            )
                        # partially-masked diagonal band
                        b_lo = max(lo, kc * P - start)
                        b_hi = min(qt * 512 + 512, kc * P - start + P)
                        bw = b_hi - b_lo
                        for h in (h0, h1):
                            if bw > 0:
                                br = b_lo - qt * 512
                                nc.vector.tensor_add(
                                    scs[h][:, br:br + bw],
                                    scs[h][:, br:br + bw],
                                    c_maskt[:, kc, bass.ds(b_lo, bw)])
                            nc.scalar.activation(
                                expts[h][:, kc, qsl], scs[h][:, rel:],
                                mybir.ActivationFunctionType.Exp,
                            )
                for h in (h0, h1):
                    expt = expts[h]
                    outU = {qt: psA.tile([P, 512], f32, tag="ev",
                                         name="outU") for qt in range(2)}
                    den = {qt: psA.tile([P, 512], f32, tag="ev",
                                        name="den") for qt in range(2)}
                    for qt in range(2):
                        kcs = nvis[qt]

                        def rng(kc):
                            lo = max(qt * 512, kc * P - start)
                            return lo, lo - qt * 512
                        for i, kc in enumerate(kcs):
                            lo, rel = rng(kc)
                            nc.tensor.matmul(
                                den[qt][:, rel:], ones_bf[:],
                                expt[:, kc, bass.ds(lo, 512 - rel)],
                                start=(i == 0), stop=(i == len(kcs) - 1),
                            )
                        for i, kc in enumerate(kcs):
                            lo, rel = rng(kc)
                            nc.tensor.matmul(
                                outU[qt][:, rel:], t_v[:, kc, bass.ts(h, P)],
                                expt[:, kc, bass.ds(lo, 512 - rel)],
                                start=(i == 0), stop=(i == len(kcs) - 1),
                            )
                    for qt in range(2):
                        rcp = att.tile([P, 512], f32, tag="rcp",
                                       name="rcp")
                        nc.vector.reciprocal(rcp[:], den[qt])
                        nc.vector.tensor_mul(
                            t_ao[:, h, bass.ts(qt, 512)], outU[qt],
                            rcp[:])

            # ====== phase 4: output projection ======
            for m in range(DC):
                wt = att.tile([P, HH, P], bf16, tag="wo", name="wo_t")
                eng = nc.scalar if m % 2 == 0 else nc.sync
                eng.dma_start(wt[:], wo_r[:, :, bass.ts(m, P)])
                for tt in range(2):
                    ps = psA.tile([P, 512], f32, tag="ev", name="ps_o")
                    for c in range(HH):
                        nc.tensor.matmul(
                            ps, wt[:, c, :], t_ao[:, c, bass.ts(tt, 512)],
                            start=(c == 0), stop=(c == HH - 1),
                        )
                    ot = att.tile([P, 512], f32, tag="ot", name="ot",
                                  bufs=3)
                    nc.vector.tensor_copy(ot[:], ps)
                    nc.sync.dma_start(outt_r[:, m, bass.ts(tt, 512)], ot[:])

    nc.compile()
    return nc


_CACHE = {}


def _get_nc(start: int):
    if start not in _CACHE:
        _CACHE[start] = build_nc(start)
    return _CACHE[start]


def _prep_inputs(X, base_freq, Wqd, bqd, gq, Wqu, bqu, Wkv, bkv, gkv,
                 Wkvu, bkvu, Wo, bo, start):
    f = np.float32
    X = np.asarray(X, f)
    base_freq = np.asarray(base_freq, f)
    Wqd = np.asarray(Wqd, f); bqd = np.asarray(bqd, f)
    gq = np.asarray(gq, f); Wqu = np.asarray(Wqu, f); bqu = np.asarray(bqu, f)
    Wkv = np.asarray(Wkv, f); bkv = np.asarray(bkv, f)
    gkv = np.asarray(gkv, f); Wkvu = np.asarray(Wkvu, f)
    bkvu = np.asarray(bkvu, f)
    Wo = np.asarray(Wo, f); bo = np.asarray(bo, f)
    start = int(np.asarray(start).item())
    assert start >= 0

    scale = QKH ** (-0.5)
    bf = ml_dtypes.bfloat16

    # qd down W split 6/6 across the TP pair; kv down duplicated
    wqd_t = Wqd.T.astype(f)                                   # (D, QL)
    wkv_t = Wkv.T.astype(f)                                   # (D, NKV)
    wqdl, bql = [], []
    for g in range(2):
        wqdl.append(np.ascontiguousarray(
            wqd_t[:, g * 768:(g + 1) * 768]).astype(bf))
        bql.append(np.ascontiguousarray(
            bqd[g * 768:(g + 1) * 768].reshape(6, P).T))
    wkvd = np.concatenate([wkv_t[:, :576], np.zeros((D, 64), f)], 1)
    wkvd = np.ascontiguousarray(wkvd).astype(bf)
    bkvd_p = np.zeros((5 * P,), f); bkvd_p[:NKV] = bkv
    bkvd = np.ascontiguousarray(bkvd_p.reshape(5, P).T)

    ang = base_freq[:S]                                       # (S, ROPE)
    cos = np.ascontiguousarray(np.cos(ang).T.astype(f))       # (ROPE, S)
    sin = np.ascontiguousarray(np.sin(ang).T.astype(f))
    cos2 = np.ascontiguousarray(np.concatenate([cos, cos], 0))  # (128, S)
    sgn = np.ones((ROPE, 1), f); sgn[:ROPE // 2] = -1.0
    sins = sin * sgn                                          # sign-folded
    sina = np.ascontiguousarray(np.concatenate([sins, sins], 0))

    # transposed additive mask: maskt[k_local, kc, q] for scoresT tiles
    bfd = ml_dtypes.bfloat16
    maskt = np.zeros((8, P, S), np.float32)
    q_glob = np.arange(S)
    for kc in range(8):
        k_glob = kc * P + np.arange(P)
        vis = k_glob[:, None] <= (start + q_glob[None, :])
        maskt[kc] = np.where(vis, 0.0, NEG)
    maskt = np.ascontiguousarray(maskt.transpose(1, 0, 2)).astype(bfd)

    # per head-group tensors
    perm_q = np.concatenate(
        [np.arange(h * QKH, h * QKH + NOPE) for h in range(HH)]
        + [np.arange(h * QKH + NOPE, (h + 1) * QKH) for h in range(HH)]
    )
    perm_kv = np.concatenate(
        [np.arange(h * (NOPE + VH), h * (NOPE + VH) + NOPE) for h in range(HH)]
        + [np.arange(h * (NOPE + VH) + NOPE, (h + 1) * (NOPE + VH))
           for h in range(HH)]
    )
    gmaps = []
    for g in range(2):
        rq = slice(g * HH * QKH, (g + 1) * HH * QKH)
        rkv = slice(g * HH * (NOPE + VH), (g + 1) * HH * (NOPE + VH))
        wqu_g = (Wqu[rq, :] * gq[None, :] * scale)[perm_q]    # (1536, QL)
        bqu_g = (bqu[rq] * scale)[perm_q]
        wkvu_g = (Wkvu[rkv, :] * gkv[None, :])[perm_kv]       # (2048, KVL)
        bkvu_g = bkvu[rkv][perm_kv]
        wo_g = Wo[:, g * HH * VH:(g + 1) * HH * VH]           # (D, 1024)
        gmaps.append({
            "wqu": np.ascontiguousarray(wqu_g.T).astype(bf),
            "bqu": np.ascontiguousarray(bqu_g.reshape(QC, P).T),
            "wkvu": np.ascontiguousarray(wkvu_g.T).astype(bf),
            "bkvuk": np.ascontiguousarray(
                bkvu_g[:HH * NOPE].reshape(HH, P).T),
            "bkvuv": np.ascontiguousarray(
                bkvu_g[HH * NOPE:].reshape(1, HH * VH)).astype(bf),
            "wo": np.ascontiguousarray(wo_g.T).astype(bf),    # (1024, D)
        })

    xts = [np.ascontiguousarray(X[b].T).astype(bf) for b in range(B)]

    in_maps = []
    for c in range(8):
        b, g = c // 2, c % 2
        m = {
            "xt": xts[b], "wqdl": wqdl[g], "bql": bql[g],
            "wkvd": wkvd, "bkvd": bkvd,
            "cos2": cos2, "sina": sina, "maskt": maskt,
            "onescol": np.ones((P, P), f),
        }
        m.update(gmaps[g])
        in_maps.append(m)
    return in_maps, bo, start


def kernel(**inputs) -> np.ndarray:
    in_maps, bo, start = _prep_inputs(**inputs)
    nc = _get_nc(start)
    try:
        res = run_bass_kernel_spmd(nc, in_maps, core_ids=list(range(8)))
    except Exception:
        res = run_bass_kernel_spmd(nc, in_maps, core_ids=list(range(8)))
    out = np.empty((B, S, D), np.float32)
    for b in range(B):
        acc = res.results[2 * b]["outt"] + res.results[2 * b + 1]["outt"]
        out[b] = acc.T + bo[None, :]
    return out
